# revision 21
# baseline (speedup 1.0000x reference)
"""MACE node-message block on 8 Trainium2 NeuronCores.

Strategy (receiver-sharded, no collectives):
  - Host sorts edges by receiver node and assigns each of the 8 cores a
    contiguous range of 1250 receiver nodes (+ its incoming edges).
  - Host gathers sender features per edge (np.take), transposes layouts,
    pads each core's edges into NCHUNK node-chunks (<=64 nodes) x TPC
    tiles of 128 edges.
  - The per-tile scatter matrices H[e, j*64+n] = y_j[e] * (rel[e]==n)
    (plus an unscaled one-hot block) are built ON DEVICE by GPSIMD
    local_scatter from (rel, y) -- 15 bytes/edge instead of 1KB dense H.
  - Device per tile (128 edges): up-project gathered feats + radial-MLP
    last layer into one 2-bank PSUM tile, single ACT evacuation, product
    blocks on DVE (with the m0b path pre-contracted: q3 = tpw3 * (v.y1)),
    then 6 PE matmuls accumulating messages [c, n] in a single PSUM bank
    per chunk (double-buffered across chunks so the PE never stalls).
  - Radial MLP runs the whole chunk (2x512 edges) packed to the full 128
    partitions via tile_position row/col tiling (full-width silu).
  - Chunk epilogues (output linear) are deferred into the middle of the
    NEXT chunk's tile loop, off the PE critical path.
  - Host reassembles the full [10000, 512] output.
"""

import numpy as np
import ml_dtypes

# ---- problem constants (hardcoded; kernel.py must be self-contained) ----
N_NODES = 10000
E_EDGES = 160000
C = 128
RB = 8
HID = 64
AVG_NEIGH = 16.0

C_000 = float(np.sqrt(0.5))
C_110 = float(np.sqrt(0.5) / np.sqrt(3.0))
C_011 = float(np.sqrt(1.5) / np.sqrt(3.0))
C_101 = float(np.sqrt(1.5) / np.sqrt(3.0))

NCORES = 8
NODES_PER_CORE = N_NODES // NCORES  # 1250
NCHUNK = 20            # node-chunks per core (<=NNODE nodes each)
NNODE = 64             # max nodes per chunk
TPC = 8                # tiles of 128 edges per chunk
CHUNK_SLOTS = TPC * 128   # 1024 edge slots per chunk
EPAD = NCHUNK * CHUNK_SLOTS
NTILES = NCHUNK * TPC
SUPER = 4              # tiles per supertile (512 edges)

BF16 = ml_dtypes.bfloat16


# --------------------------------------------------------------------------
# Host-side sharding / layout preparation
# --------------------------------------------------------------------------

def _host_prep(node_feats, edge_attrs, edge_feats, edge_index):
    sender = edge_index[0].astype(np.int64)
    receiver = edge_index[1].astype(np.int64)
    deg = np.bincount(receiver, minlength=N_NODES)
    order = np.argsort(receiver, kind="stable")
    node_edge_start = np.concatenate([[0], np.cumsum(deg)])

    per_core = []
    for c in range(NCORES):
        lo, hi = NODES_PER_CORE * c, NODES_PER_CORE * (c + 1)
        # greedy chunking: <=NNODE nodes and <=CHUNK_SLOTS edges per chunk
        chunks = []  # (node_start, node_end, edge_count)
        n = lo
        while n < hi:
            start = n
            ec = 0
            while n < hi and (n - start) < NNODE and ec + deg[n] <= CHUNK_SLOTS:
                ec += deg[n]
                n += 1
            chunks.append((start, n, int(ec)))
        assert len(chunks) <= NCHUNK, (
            f"core {c}: needs {len(chunks)} chunks > NCHUNK={NCHUNK}"
        )
        while len(chunks) < NCHUNK:
            chunks.append((hi, hi, 0))

        slot_sender = np.zeros(EPAD, np.int64)
        slot_rel = np.zeros(EPAD, np.int64)
        slot_y = np.zeros((EPAD, 4), np.float32)
        slot_ef = np.zeros((EPAD, RB), np.float32)
        used = np.zeros(EPAD, np.float32)
        for k, (s, e, ec) in enumerate(chunks):
            if ec == 0:
                continue
            seg = order[node_edge_start[s]: node_edge_start[s] + ec]
            base = k * CHUNK_SLOTS
            slot_sender[base: base + ec] = sender[seg]
            slot_rel[base: base + ec] = receiver[seg] - s
            slot_y[base: base + ec] = edge_attrs[seg]
            slot_ef[base: base + ec] = edge_feats[seg]
            used[base: base + ec] = 1.0

        # gathered sender feats, transposed per block: [t, cin, blk, e]
        # block 4 = gvy(e, cin) = sum_m v_m(e, cin) * y1m(e)  (host-folded,
        # like the y-scaled scatter matrices; Wv^T gvy = v . y1 up-projected)
        g = node_feats[slot_sender]                      # [EPAD, 512]
        s_blk = g[:, :C]
        v = g[:, C:].reshape(EPAD, C, 3)
        gvy = np.einsum('ecm,em->ec', v, slot_y[:, 1:4]).astype(np.float32)
        blocks = np.stack([s_blk, v[:, :, 0], v[:, :, 1], v[:, :, 2], gvy],
                          axis=1)
        gfeat = np.ascontiguousarray(
            blocks.reshape(NTILES, 128, 5, C).transpose(0, 3, 2, 1)
        ).astype(BF16)
        g4 = np.ascontiguousarray(
            gfeat.reshape(NTILES // SUPER, SUPER, 128, 5 * C)
            .transpose(0, 2, 1, 3))

        # H-build inputs: per chunk [128 part(e-in-tile), TPC, 6]
        # blocks j=0..3: H_j[e, j*64+rel] = y_j(e); block 4: unscaled
        # one-hot mask[e, 4*64+rel] = used(e); col 5 unused (-1).
        rel_t = slot_rel.reshape(NCHUNK, TPC, 128)      # [k, t, e]
        y_t = slot_y.reshape(NCHUNK, TPC, 128, 4)       # [k, t, e, 4]
        used_t = used.reshape(NCHUNK, TPC, 128)
        relidx = np.full((NCHUNK, TPC, 128, 6), -1, np.int16)
        relidx[:, :, :, :5] = (np.arange(5)[None, None, None, :] * NNODE
                               + rel_t[:, :, :, None])
        relidx = np.ascontiguousarray(relidx.transpose(0, 2, 1, 3))
        y6 = np.zeros((NCHUNK, TPC, 128, 6), np.float32)
        y6[:, :, :, :4] = y_t
        y6[:, :, :, 4] = used_t
        y6 = np.ascontiguousarray(y6.transpose(0, 2, 1, 3)).astype(BF16)

        # radial basis, transposed per chunk: [k, r, slot]
        eft = np.ascontiguousarray(
            slot_ef.reshape(NCHUNK, CHUNK_SLOTS, RB).transpose(0, 2, 1)
        ).astype(BF16)

        per_core.append({
            "chunks": chunks,
            "gfeat": g4,
            "relidx": relidx,
            "y6": y6,
            "eft": eft,
        })
    return per_core


def _weights_prep(W_up_s, W_up_v, W_mlp1, W_mlp2, W_mlp3, W_mlp4,
                  W_lin_s, W_lin_v):
    su = 1.0 / np.sqrt(np.float32(C))
    wup = np.stack([W_up_s * su, W_up_v * su], axis=1).astype(BF16)  # [cin,2,cout]

    w1 = (W_mlp1 / np.sqrt(np.float32(RB))).astype(BF16)
    # hidden-layer weights duplicated on both partition halves for the
    # tile_position-packed MLP (rows 0:64 and 64:128 hold the same matrix)
    w2d = np.concatenate([(W_mlp2 / np.sqrt(np.float32(HID)))] * 2,
                         axis=0).astype(BF16)           # [128, HID]
    w3d = np.concatenate([(W_mlp3 / np.sqrt(np.float32(HID)))] * 2,
                         axis=0).astype(BF16)
    # per-path constants folded into the last MLP layer (tpw order 0,1,2,3)
    scales = np.array([C_000, C_011, C_101, C_110], np.float32)
    w4p = ((W_mlp4 / np.sqrt(np.float32(HID))).reshape(HID, 4, C)
           * scales[None, :, None]).reshape(HID, 4 * C)
    w4pd = np.concatenate([w4p] * 2, axis=0).astype(BF16)  # [128, 4C]

    sl = 1.0 / (np.sqrt(np.float32(2 * C)) * AVG_NEIGH)
    wls = W_lin_s * sl   # [256, 128]
    wlv = W_lin_v * sl
    wl = np.stack([wls[:C], wls[C:], wlv[:C], wlv[C:]], axis=1).astype(np.float32)
    return {"wup": wup, "w1": w1, "w2d": w2d, "w3d": w3d, "w4pd": w4pd,
            "wl": wl}


# --------------------------------------------------------------------------
# Device program
# --------------------------------------------------------------------------

def build_program():
    import concourse.bacc as bacc
    import concourse.mybir as mybir
    import concourse.tile as tile

    f32 = mybir.dt.float32
    bf16 = mybir.dt.bfloat16
    i16 = mybir.dt.int16
    MUL = mybir.AluOpType.mult
    ADD = mybir.AluOpType.add

    nc = bacc.Bacc(None, target_bir_lowering=False)

    NSUPER = NTILES // SUPER
    gfeatD = nc.dram_tensor("gfeat", [NSUPER, 128, SUPER, 640], bf16,
                            kind="ExternalInput")
    relidxD = nc.dram_tensor("relidx", [NCHUNK, 128, TPC, 6], i16,
                             kind="ExternalInput")
    y6D = nc.dram_tensor("y6", [NCHUNK, 128, TPC, 6], bf16,
                         kind="ExternalInput")
    eftD = nc.dram_tensor("eft", [NCHUNK, RB, CHUNK_SLOTS], bf16,
                          kind="ExternalInput")
    wupD = nc.dram_tensor("wup", [128, 2, 128], bf16, kind="ExternalInput")
    w1D = nc.dram_tensor("w1", [RB, HID], bf16, kind="ExternalInput")
    w2dD = nc.dram_tensor("w2d", [128, HID], bf16, kind="ExternalInput")
    w3dD = nc.dram_tensor("w3d", [128, HID], bf16, kind="ExternalInput")
    w4pdD = nc.dram_tensor("w4pd", [128, 4 * C], bf16, kind="ExternalInput")
    wlD = nc.dram_tensor("wl", [128, 4, 128], f32, kind="ExternalInput")
    outD = nc.dram_tensor("outb", [NCHUNK * NNODE, 512], f32,
                          kind="ExternalOutput")

    SILU = mybir.ActivationFunctionType.Silu
    COPYF = mybir.ActivationFunctionType.Copy

    with tile.TileContext(nc) as tc:
        with (
            tc.tile_pool(name="const", bufs=1) as cp,
            tc.tile_pool(name="work", bufs=4) as wp,
            tc.tile_pool(name="hpool", bufs=4) as hp,
            tc.tile_pool(name="hs", bufs=2) as hsp,
            tc.tile_pool(name="chk", bufs=2) as chp,
            tc.tile_pool(name="ps", bufs=2, space="PSUM") as ps,
            tc.tile_pool(name="misc", bufs=1, space="PSUM") as mp,
            tc.tile_pool(name="msgp", bufs=1, space="PSUM") as msgp,
        ):
            # constants
            wupS = cp.tile([128, 2, 128], bf16, tag="wup")
            w1S = cp.tile([RB, HID], bf16, tag="w1")
            w2dS = cp.tile([128, HID], bf16, tag="w2d")
            w3dS = cp.tile([128, HID], bf16, tag="w3d")
            w4pdS = cp.tile([128, 4 * C], bf16, tag="w4pd")
            wlS = cp.tile([128, 4, 128], f32, tag="wl")
            nc.sync.dma_start(out=wupS[:], in_=wupD[:])
            nc.sync.dma_start(out=w1S[:], in_=w1D[:])
            nc.sync.dma_start(out=w2dS[:], in_=w2dD[:])
            nc.sync.dma_start(out=w3dS[:], in_=w3dD[:])
            nc.sync.dma_start(out=w4pdS[:], in_=w4pdD[:])
            nc.sync.dma_start(out=wlS[:], in_=wlD[:])

            def emit_epi(k, msgS):
                # output linear for chunk k from its SBUF message copy.
                # msgS blocks: [v1x, v1y, v1z, s1, v2x, v2y, v2z, s2]
                outPF = mp.tile([128, 512], f32, tag="scr")
                outP = outPF[0:NNODE, :]
                nc.tensor.matmul(out=outP[:, 0:128], lhsT=msgS[:, 3, :],
                                 rhs=wlS[:, 0, :], start=True, stop=False)
                nc.tensor.matmul(out=outP[:, 0:128], lhsT=msgS[:, 7, :],
                                 rhs=wlS[:, 1, :], start=False, stop=False)
                for m in range(3):
                    osl = slice((1 + m) * 128, (2 + m) * 128)
                    nc.tensor.matmul(out=outP[:, osl],
                                     lhsT=msgS[:, m, :],
                                     rhs=wlS[:, 2, :], start=False,
                                     stop=False)
                    nc.tensor.matmul(out=outP[:, osl],
                                     lhsT=msgS[:, 4 + m, :],
                                     rhs=wlS[:, 3, :], start=False,
                                     stop=m == 2)
                outS = chp.tile([NNODE, 512], f32, tag="outS")
                nc.vector.tensor_copy(out=outS[:], in_=outP[:])
                nc.sync.dma_start(out=outD[k * NNODE:(k + 1) * NNODE, :],
                                  in_=outS[:])

            prev = None  # (k, msgS) awaiting epilogue
            for k in range(NCHUNK):
                efS = chp.tile([RB, CHUNK_SLOTS], bf16, tag="ef")
                nc.sync.dma_start(out=efS[:], in_=eftD[k])
                relS = chp.tile([128, TPC, 6], i16, tag="rel")
                nc.sync.dma_start(out=relS[:], in_=relidxD[k])
                y6S = chp.tile([128, TPC, 6], bf16, tag="y6")
                nc.sync.dma_start(out=y6S[:], in_=y6D[k])

                # message accumulator, [cin-block, n] orientation, 1 bank:
                # blocks [v1x, v1y, v1z, s1, v2x, v2y, v2z, s2]
                msgAB = msgp.tile([128, 8, NNODE], f32, tag="msgAB")

                # ---- radial MLP: whole chunk (2x512 edges) packed to 128
                # partitions via tile_position row/col tiling ----
                ea = slice(0, 512)
                eb = slice(512, 1024)
                h1p = mp.tile([128, 512], f32, tag="scr")
                nc.tensor.matmul(out=h1p[0:64, :], lhsT=w1S[:],
                                 rhs=efS[:, ea], start=True, stop=True,
                                 tile_position=(0, 0))
                nc.tensor.matmul(out=h1p[64:128, :], lhsT=w1S[:],
                                 rhs=efS[:, eb], start=True, stop=True,
                                 tile_position=(0, 64))
                h1s = hsp.tile([128, 512], bf16, tag="h1s")
                nc.scalar.activation(out=h1s[:], in_=h1p[:], func=SILU)

                h2p = mp.tile([128, 512], f32, tag="scr")
                nc.tensor.matmul(out=h2p[0:64, :], lhsT=w2dS[0:64, :],
                                 rhs=h1s[0:64, :], start=True, stop=True,
                                 tile_position=(0, 0))
                nc.tensor.matmul(out=h2p[64:128, :], lhsT=w2dS[64:128, :],
                                 rhs=h1s[64:128, :], start=True,
                                 stop=True, tile_position=(64, 64))
                h2s = hsp.tile([128, 512], bf16, tag="h2s")
                nc.scalar.activation(out=h2s[:], in_=h2p[:], func=SILU)

                h3p = mp.tile([128, 512], f32, tag="scr")
                nc.tensor.matmul(out=h3p[0:64, :], lhsT=w3dS[0:64, :],
                                 rhs=h2s[0:64, :], start=True, stop=True,
                                 tile_position=(0, 0))
                nc.tensor.matmul(out=h3p[64:128, :], lhsT=w3dS[64:128, :],
                                 rhs=h2s[64:128, :], start=True,
                                 stop=True, tile_position=(64, 64))
                h3s = hsp.tile([128, 512], bf16, tag="h3s")
                nc.scalar.activation(out=h3s[:], in_=h3p[:], func=SILU)

                for u in range(TPC):
                    sg = k * 2 + u // SUPER         # global supertile index
                    uu = u % SUPER                  # tile within supertile
                    half = u // SUPER               # which partition half
                    first = u == 0
                    last = u == TPC - 1

                    if uu == 0:
                        g4 = wp.tile([128, SUPER, 640], bf16, tag="g")
                        nc.sync.dma_start(out=g4[:], in_=gfeatD[sg])
                    gS = g4[:, uu, :]

                    # ---- H build on GPSIMD: H[e, j*64+rel] = y_j, plus
                    # unscaled one-hot mask in block 4 ----
                    hS = hp.tile([128, 5 * NNODE], bf16, tag="hm")
                    nc.gpsimd.local_scatter(
                        out_ap=hS[:], data_ap=y6S[:, u, :],
                        idxs_ap=relS[:, u, :], channels=128,
                        num_elems=5 * NNODE, num_idxs=6)

                    # ---- up-projection + tpw into one 3-bank PSUM tile:
                    # cols 0:512 feat (S, V1..V3), 512:1024 tpw (t0..t3),
                    # 1024:1152 VD = Wv^T gvy (= v.y1 up-projected).
                    # tpw first so its ACT evacuation overlaps the up MMs.
                    ftP = ps.tile([128, 9, 128], f32, tag="ftp")
                    hrow = slice(64 * half, 64 * half + 64)
                    nc.tensor.matmul(
                        out=ftP[:, 4:8, :],
                        lhsT=h3s[hrow, uu * 128:(uu + 1) * 128],
                        rhs=w4pdS[hrow, :], start=True, stop=True)
                    nc.tensor.matmul(
                        out=ftP[:, 8, :],
                        lhsT=gS[:, 512:640],
                        rhs=wupS[:, 1, :], start=True, stop=True)
                    for b in range(4):
                        nc.tensor.matmul(
                            out=ftP[:, b, :],
                            lhsT=gS[:, b * 128:(b + 1) * 128],
                            rhs=wupS[:, min(b, 1), :],
                            start=b == 0, stop=b == 3)

                    # deferred epilogue of the previous chunk, issued into
                    # the middle of this chunk's PE stream
                    if u == 3 and prev is not None:
                        emit_epi(*prev)
                        prev = None

                    # evacuate only the tpw blocks; products read the
                    # feat blocks directly from PSUM bank 0 (different
                    # bank than the ACT read, so they run in parallel)
                    ftS = wp.tile([128, 4, 128], bf16, tag="ft")
                    nc.scalar.activation(out=ftS[:], in_=ftP[:, 4:8, :],
                                         func=COPYF)
                    tpwS = ftS

                    # ---- elementwise product blocks (DVE, bf16) ----
                    # slots: p0, p1, p2x, p2y, p2z, q3; 6/7 scratch
                    prodS = wp.tile([128, 8, 128], bf16, tag="prod")
                    nc.vector.tensor_tensor(
                        out=prodS[:, 0:2, :], in0=tpwS[:, 0:2, :],
                        in1=ftP[:, 0:1, :].broadcast_to([128, 2, 128]),
                        op=MUL)
                    nc.vector.tensor_tensor(
                        out=prodS[:, 2:5, :],
                        in0=tpwS[:, 2:3, :].broadcast_to([128, 3, 128]),
                        in1=ftP[:, 1:4, :], op=MUL)
                    # q3 = t3 * (v.y1); the second operand comes straight
                    # from PSUM bank 3 of ftP (different bank than the cast)
                    nc.vector.tensor_tensor(
                        out=prodS[:, 5, :], in0=tpwS[:, 3, :],
                        in1=ftP[:, 8, :], op=MUL)

                    # ---- weighted segment-sum (product stationary) ----
                    # msgAB blocks [v1x, v1y, v1z, s1, v2x, v2y, v2z, s2]
                    nc.tensor.matmul(out=msgAB[:, 0:3, :],
                                     lhsT=prodS[:, 1, :],
                                     rhs=hS[:, NNODE:4 * NNODE],
                                     start=first, stop=False)
                    nc.tensor.matmul(out=msgAB[:, 3, :],
                                     lhsT=prodS[:, 0, :],
                                     rhs=hS[:, 0:NNODE],
                                     start=False, stop=False)
                    for m in range(3):
                        nc.tensor.matmul(
                            out=msgAB[:, 4 + m, :],
                            lhsT=prodS[:, 2 + m, :],
                            rhs=hS[:, 0:NNODE],
                            start=False, stop=False)
                    nc.tensor.matmul(out=msgAB[:, 7, :],
                                     lhsT=prodS[:, 5, :],
                                     rhs=hS[:, 4 * NNODE:5 * NNODE],
                                     start=False, stop=last)

                # copy accumulated messages to SBUF; epilogue deferred
                msgS = chp.tile([128, 8, NNODE], f32, tag="msgS")
                nc.vector.tensor_copy(out=msgS[:], in_=msgAB[:])
                prev = (k, msgS)

            emit_epi(*prev)

    nc.compile()
    return nc


# --------------------------------------------------------------------------
# Entry point
# --------------------------------------------------------------------------

def _assemble(results, per_core):
    out = np.zeros((N_NODES, 512), np.float32)
    for c in range(NCORES):
        ob = results[c]["outb"]
        for k, (s, e, _ec) in enumerate(per_core[c]["chunks"]):
            w = e - s
            if w == 0:
                continue
            rows = ob[k * NNODE: k * NNODE + w]
            out[s:e, :C] = rows[:, :C]
            out[s:e, C:] = np.stack(
                [rows[:, C:2 * C], rows[:, 2 * C:3 * C], rows[:, 3 * C:]],
                axis=2).reshape(w, 3 * C)
    return out


def run(inputs, trace=False, **kwargs):
    from concourse.bass_utils import run_bass_kernel_spmd

    per_core = _host_prep(inputs["node_feats"], inputs["edge_attrs"],
                          inputs["edge_feats"], inputs["edge_index"])
    wts = _weights_prep(inputs["W_up_s"], inputs["W_up_v"], inputs["W_mlp1"],
                        inputs["W_mlp2"], inputs["W_mlp3"], inputs["W_mlp4"],
                        inputs["W_lin_s"], inputs["W_lin_v"])
    in_maps = [
        {"gfeat": pc["gfeat"], "relidx": pc["relidx"], "y6": pc["y6"],
         "eft": pc["eft"], **wts}
        for pc in per_core
    ]
    nc = build_program()
    res = run_bass_kernel_spmd(nc, in_maps, core_ids=list(range(NCORES)),
                               trace=trace, **kwargs)
    return _assemble(res.results, per_core), res


def kernel(**inputs):
    return run(inputs)[0]


# revision 22
# speedup vs baseline: 1.1753x; 1.1753x over previous
"""MACE node-message block on 8 Trainium2 NeuronCores.

Strategy (receiver-sharded, no collectives):
  - Host sorts edges by receiver node and assigns each of the 8 cores a
    contiguous range of 1250 receiver nodes (+ its incoming edges).
  - Host gathers sender features per edge (np.take), transposes layouts,
    pads each core's edges into NCHUNK node-chunks (<=64 nodes) x TPC
    tiles of 128 edges.
  - The per-tile scatter matrices H[e, j*64+n] = y_j[e] * (rel[e]==n)
    (plus an unscaled one-hot block) are built ON DEVICE by GPSIMD
    local_scatter from (rel, y) -- 15 bytes/edge instead of 1KB dense H.
  - Device per tile (128 edges): up-project gathered feats + radial-MLP
    last layer into one 2-bank PSUM tile, single ACT evacuation, product
    blocks on DVE (with the m0b path pre-contracted: q3 = tpw3 * (v.y1)),
    then 6 PE matmuls accumulating messages [c, n] in a single PSUM bank
    per chunk (double-buffered across chunks so the PE never stalls).
  - Radial MLP runs the whole chunk (2x512 edges) packed to the full 128
    partitions via tile_position row/col tiling (full-width silu).
  - Chunk epilogues (output linear) are deferred into the middle of the
    NEXT chunk's tile loop, off the PE critical path.
  - Host reassembles the full [10000, 512] output.
"""

import numpy as np
import ml_dtypes

# ---- problem constants (hardcoded; kernel.py must be self-contained) ----
N_NODES = 10000
E_EDGES = 160000
C = 128
RB = 8
HID = 64
AVG_NEIGH = 16.0

C_000 = float(np.sqrt(0.5))
C_110 = float(np.sqrt(0.5) / np.sqrt(3.0))
C_011 = float(np.sqrt(1.5) / np.sqrt(3.0))
C_101 = float(np.sqrt(1.5) / np.sqrt(3.0))

NCORES = 8
NODES_PER_CORE = N_NODES // NCORES  # 1250
NCHUNK = 20            # node-chunks per core (<=NNODE nodes each)
NNODE = 64             # max nodes per chunk
TPC = 8                # tiles of 128 edges per chunk
CHUNK_SLOTS = TPC * 128   # 1024 edge slots per chunk
EPAD = NCHUNK * CHUNK_SLOTS
NTILES = NCHUNK * TPC
SUPER = 4              # tiles per supertile (512 edges)

BF16 = ml_dtypes.bfloat16


# --------------------------------------------------------------------------
# Host-side sharding / layout preparation
# --------------------------------------------------------------------------

def _host_prep(node_feats, edge_attrs, edge_feats, edge_index):
    sender = edge_index[0].astype(np.int64)
    receiver = edge_index[1].astype(np.int64)
    deg = np.bincount(receiver, minlength=N_NODES)
    order = np.argsort(receiver, kind="stable")
    node_edge_start = np.concatenate([[0], np.cumsum(deg)])

    per_core = []
    for c in range(NCORES):
        lo, hi = NODES_PER_CORE * c, NODES_PER_CORE * (c + 1)
        # greedy chunking: <=NNODE nodes and <=CHUNK_SLOTS edges per chunk
        chunks = []  # (node_start, node_end, edge_count)
        n = lo
        while n < hi:
            start = n
            ec = 0
            while n < hi and (n - start) < NNODE and ec + deg[n] <= CHUNK_SLOTS:
                ec += deg[n]
                n += 1
            chunks.append((start, n, int(ec)))
        assert len(chunks) <= NCHUNK, (
            f"core {c}: needs {len(chunks)} chunks > NCHUNK={NCHUNK}"
        )
        while len(chunks) < NCHUNK:
            chunks.append((hi, hi, 0))

        slot_sender = np.zeros(EPAD, np.int64)
        slot_rel = np.zeros(EPAD, np.int64)
        slot_y = np.zeros((EPAD, 4), np.float32)
        slot_ef = np.zeros((EPAD, RB), np.float32)
        used = np.zeros(EPAD, np.float32)
        for k, (s, e, ec) in enumerate(chunks):
            if ec == 0:
                continue
            seg = order[node_edge_start[s]: node_edge_start[s] + ec]
            base = k * CHUNK_SLOTS
            slot_sender[base: base + ec] = sender[seg]
            slot_rel[base: base + ec] = receiver[seg] - s
            slot_y[base: base + ec] = edge_attrs[seg]
            slot_ef[base: base + ec] = edge_feats[seg]
            used[base: base + ec] = 1.0

        # gathered sender feats, transposed per block: [t, cin, blk, e]
        # block 4 = gvy(e, cin) = sum_m v_m(e, cin) * y1m(e)  (host-folded,
        # like the y-scaled scatter matrices; Wv^T gvy = v . y1 up-projected)
        g = node_feats[slot_sender]                      # [EPAD, 512]
        s_blk = g[:, :C]
        v = g[:, C:].reshape(EPAD, C, 3)
        gvy = np.einsum('ecm,em->ec', v, slot_y[:, 1:4]).astype(np.float32)
        blocks = np.stack([s_blk, v[:, :, 0], v[:, :, 1], v[:, :, 2], gvy],
                          axis=1)
        gfeat = np.ascontiguousarray(
            blocks.reshape(NTILES, 128, 5, C).transpose(0, 3, 2, 1)
        ).astype(BF16)
        g4 = np.ascontiguousarray(
            gfeat.reshape(NTILES // SUPER, SUPER, 128, 5 * C)
            .transpose(0, 2, 1, 3))

        # H-build inputs: per chunk [128 part(e-in-tile), TPC, 6]
        # blocks j=0..3: H_j[e, j*64+rel] = y_j(e); block 4: unscaled
        # one-hot mask[e, 4*64+rel] = used(e); col 5 unused (-1).
        rel_t = slot_rel.reshape(NCHUNK, TPC, 128)      # [k, t, e]
        y_t = slot_y.reshape(NCHUNK, TPC, 128, 4)       # [k, t, e, 4]
        used_t = used.reshape(NCHUNK, TPC, 128)
        relidx = np.full((NCHUNK, TPC, 128, 6), -1, np.int16)
        relidx[:, :, :, :5] = (np.arange(5)[None, None, None, :] * NNODE
                               + rel_t[:, :, :, None])
        relidx = np.ascontiguousarray(relidx.transpose(0, 2, 1, 3))
        y6 = np.zeros((NCHUNK, TPC, 128, 6), np.float32)
        y6[:, :, :, :4] = y_t
        y6[:, :, :, 4] = used_t
        y6 = np.ascontiguousarray(y6.transpose(0, 2, 1, 3)).astype(BF16)

        # radial basis, transposed per chunk: [k, r, slot]
        eft = np.ascontiguousarray(
            slot_ef.reshape(NCHUNK, CHUNK_SLOTS, RB).transpose(0, 2, 1)
        ).astype(BF16)

        per_core.append({
            "chunks": chunks,
            "gfeat": g4,
            "relidx": relidx,
            "y6": y6,
            "eft": eft,
        })
    return per_core


def _weights_prep(W_up_s, W_up_v, W_mlp1, W_mlp2, W_mlp3, W_mlp4,
                  W_lin_s, W_lin_v):
    su = 1.0 / np.sqrt(np.float32(C))
    wup = np.stack([W_up_s * su, W_up_v * su], axis=1).astype(BF16)  # [cin,2,cout]

    w1 = (W_mlp1 / np.sqrt(np.float32(RB))).astype(BF16)
    # hidden-layer weights duplicated on both partition halves for the
    # tile_position-packed MLP (rows 0:64 and 64:128 hold the same matrix)
    w2d = np.concatenate([(W_mlp2 / np.sqrt(np.float32(HID)))] * 2,
                         axis=0).astype(BF16)           # [128, HID]
    w3d = np.concatenate([(W_mlp3 / np.sqrt(np.float32(HID)))] * 2,
                         axis=0).astype(BF16)
    # per-path constants folded into the last MLP layer (tpw order 0,1,2,3)
    scales = np.array([C_000, C_011, C_101, C_110], np.float32)
    w4p = ((W_mlp4 / np.sqrt(np.float32(HID))).reshape(HID, 4, C)
           * scales[None, :, None]).reshape(HID, 4 * C)
    w4pd = np.concatenate([w4p] * 2, axis=0).astype(BF16)  # [128, 4C]

    sl = 1.0 / (np.sqrt(np.float32(2 * C)) * AVG_NEIGH)
    wls = W_lin_s * sl   # [256, 128]
    wlv = W_lin_v * sl
    wl = np.stack([wls[:C], wls[C:], wlv[:C], wlv[C:]], axis=1).astype(np.float32)
    return {"wup": wup, "w1": w1, "w2d": w2d, "w3d": w3d, "w4pd": w4pd,
            "wl": wl}


# --------------------------------------------------------------------------
# Device program
# --------------------------------------------------------------------------

def build_program():
    import concourse.bacc as bacc
    import concourse.mybir as mybir
    import concourse.tile as tile

    f32 = mybir.dt.float32
    bf16 = mybir.dt.bfloat16
    i16 = mybir.dt.int16
    MUL = mybir.AluOpType.mult
    ADD = mybir.AluOpType.add

    nc = bacc.Bacc(None, target_bir_lowering=False)

    NSUPER = NTILES // SUPER
    gfeatD = nc.dram_tensor("gfeat", [NSUPER, 128, SUPER, 640], bf16,
                            kind="ExternalInput")
    relidxD = nc.dram_tensor("relidx", [NCHUNK, 128, TPC, 6], i16,
                             kind="ExternalInput")
    y6D = nc.dram_tensor("y6", [NCHUNK, 128, TPC, 6], bf16,
                         kind="ExternalInput")
    eftD = nc.dram_tensor("eft", [NCHUNK, RB, CHUNK_SLOTS], bf16,
                          kind="ExternalInput")
    wupD = nc.dram_tensor("wup", [128, 2, 128], bf16, kind="ExternalInput")
    w1D = nc.dram_tensor("w1", [RB, HID], bf16, kind="ExternalInput")
    w2dD = nc.dram_tensor("w2d", [128, HID], bf16, kind="ExternalInput")
    w3dD = nc.dram_tensor("w3d", [128, HID], bf16, kind="ExternalInput")
    w4pdD = nc.dram_tensor("w4pd", [128, 4 * C], bf16, kind="ExternalInput")
    wlD = nc.dram_tensor("wl", [128, 4, 128], f32, kind="ExternalInput")
    outD = nc.dram_tensor("outb", [NCHUNK * NNODE, 512], f32,
                          kind="ExternalOutput")

    SILU = mybir.ActivationFunctionType.Silu
    COPYF = mybir.ActivationFunctionType.Copy

    with tile.TileContext(nc) as tc:
        with (
            tc.tile_pool(name="const", bufs=1) as cp,
            tc.tile_pool(name="work", bufs=5) as wp,
            tc.tile_pool(name="hpool", bufs=6) as hp,
            tc.tile_pool(name="hs", bufs=2) as hsp,
            tc.tile_pool(name="chk", bufs=2) as chp,
            tc.tile_pool(name="ps", bufs=2, space="PSUM") as ps,
            tc.tile_pool(name="misc", bufs=1, space="PSUM") as mp,
            tc.tile_pool(name="msgp", bufs=1, space="PSUM") as msgp,
        ):
            # constants
            wupS = cp.tile([128, 2, 128], bf16, tag="wup")
            w1S = cp.tile([RB, HID], bf16, tag="w1")
            w2dS = cp.tile([128, HID], bf16, tag="w2d")
            w3dS = cp.tile([128, HID], bf16, tag="w3d")
            w4pdS = cp.tile([128, 4 * C], bf16, tag="w4pd")
            wlS = cp.tile([128, 4, 128], f32, tag="wl")
            nc.sync.dma_start(out=wupS[:], in_=wupD[:])
            nc.sync.dma_start(out=w1S[:], in_=w1D[:])
            nc.sync.dma_start(out=w2dS[:], in_=w2dD[:])
            nc.sync.dma_start(out=w3dS[:], in_=w3dD[:])
            nc.sync.dma_start(out=w4pdS[:], in_=w4pdD[:])
            nc.sync.dma_start(out=wlS[:], in_=wlD[:])

            def emit_epi(k, msgS):
                # output linear for chunk k from its SBUF message copy.
                # msgS blocks: [v1x, v1y, v1z, s1, v2x, v2y, v2z, s2]
                outPF = mp.tile([128, 512], f32, tag="scr")
                outP = outPF[0:NNODE, :]
                nc.tensor.matmul(out=outP[:, 0:128], lhsT=msgS[:, 3, :],
                                 rhs=wlS[:, 0, :], start=True, stop=False)
                nc.tensor.matmul(out=outP[:, 0:128], lhsT=msgS[:, 7, :],
                                 rhs=wlS[:, 1, :], start=False, stop=False)
                for m in range(3):
                    osl = slice((1 + m) * 128, (2 + m) * 128)
                    nc.tensor.matmul(out=outP[:, osl],
                                     lhsT=msgS[:, m, :],
                                     rhs=wlS[:, 2, :], start=False,
                                     stop=False)
                    nc.tensor.matmul(out=outP[:, osl],
                                     lhsT=msgS[:, 4 + m, :],
                                     rhs=wlS[:, 3, :], start=False,
                                     stop=m == 2)
                outS = chp.tile([NNODE, 512], f32, tag="outS")
                nc.vector.tensor_copy(out=outS[:], in_=outP[:])
                nc.sync.dma_start(out=outD[k * NNODE:(k + 1) * NNODE, :],
                                  in_=outS[:])

            prev = None  # (k, msgS) awaiting epilogue
            for k in range(NCHUNK):
                efS = chp.tile([RB, CHUNK_SLOTS], bf16, tag="ef")
                nc.sync.dma_start(out=efS[:], in_=eftD[k])
                relS = chp.tile([128, TPC, 6], i16, tag="rel")
                nc.sync.dma_start(out=relS[:], in_=relidxD[k])
                y6S = chp.tile([128, TPC, 6], bf16, tag="y6")
                nc.sync.dma_start(out=y6S[:], in_=y6D[k])

                # message accumulator, [cin-block, n] orientation, 1 bank:
                # blocks [v1x, v1y, v1z, s1, v2x, v2y, v2z, s2]
                msgAB = msgp.tile([128, 8, NNODE], f32, tag="msgAB")

                # ---- radial MLP: whole chunk (2x512 edges) packed to 128
                # partitions via tile_position row/col tiling ----
                ea = slice(0, 512)
                eb = slice(512, 1024)
                h1p = mp.tile([128, 512], f32, tag="scr")
                nc.tensor.matmul(out=h1p[0:64, :], lhsT=w1S[:],
                                 rhs=efS[:, ea], start=True, stop=True,
                                 tile_position=(0, 0))
                nc.tensor.matmul(out=h1p[64:128, :], lhsT=w1S[:],
                                 rhs=efS[:, eb], start=True, stop=True,
                                 tile_position=(0, 64))
                h1s = hsp.tile([128, 512], bf16, tag="h1s")
                nc.scalar.activation(out=h1s[:], in_=h1p[:], func=SILU)

                h2p = mp.tile([128, 512], f32, tag="scr")
                nc.tensor.matmul(out=h2p[0:64, :], lhsT=w2dS[0:64, :],
                                 rhs=h1s[0:64, :], start=True, stop=True,
                                 tile_position=(0, 0))
                nc.tensor.matmul(out=h2p[64:128, :], lhsT=w2dS[64:128, :],
                                 rhs=h1s[64:128, :], start=True,
                                 stop=True, tile_position=(64, 64))
                h2s = hsp.tile([128, 512], bf16, tag="h2s")
                nc.scalar.activation(out=h2s[:], in_=h2p[:], func=SILU)

                h3p = mp.tile([128, 512], f32, tag="scr")
                nc.tensor.matmul(out=h3p[0:64, :], lhsT=w3dS[0:64, :],
                                 rhs=h2s[0:64, :], start=True, stop=True,
                                 tile_position=(0, 0))
                nc.tensor.matmul(out=h3p[64:128, :], lhsT=w3dS[64:128, :],
                                 rhs=h2s[64:128, :], start=True,
                                 stop=True, tile_position=(64, 64))
                h3s = hsp.tile([128, 512], bf16, tag="h3s")
                nc.scalar.activation(out=h3s[:], in_=h3p[:], func=SILU)

                for u in range(TPC):
                    sg = k * 2 + u // SUPER         # global supertile index
                    uu = u % SUPER                  # tile within supertile
                    half = u // SUPER               # which partition half
                    first = u == 0
                    last = u == TPC - 1

                    if uu == 0:
                        g4 = wp.tile([128, SUPER, 640], bf16, tag="g")
                        nc.sync.dma_start(out=g4[:], in_=gfeatD[sg])
                    gS = g4[:, uu, :]

                    # ---- H build on GPSIMD: H[e, j*64+rel] = y_j, plus
                    # unscaled one-hot mask in block 4 ----
                    hS = hp.tile([128, 5 * NNODE], bf16, tag="hm")
                    nc.gpsimd.local_scatter(
                        out_ap=hS[:], data_ap=y6S[:, u, :],
                        idxs_ap=relS[:, u, :], channels=128,
                        num_elems=5 * NNODE, num_idxs=6)

                    # ---- up-projection + tpw into one 3-bank PSUM tile:
                    # cols 0:512 feat (S, V1..V3), 512:1024 tpw (t0..t3),
                    # 1024:1152 VD = Wv^T gvy (= v.y1 up-projected).
                    # tpw first so its ACT evacuation overlaps the up MMs.
                    ftP = ps.tile([128, 9, 128], f32, tag="ftp")
                    hrow = slice(64 * half, 64 * half + 64)
                    nc.tensor.matmul(
                        out=ftP[:, 4:8, :],
                        lhsT=h3s[hrow, uu * 128:(uu + 1) * 128],
                        rhs=w4pdS[hrow, :], start=True, stop=True)
                    nc.tensor.matmul(
                        out=ftP[:, 8, :],
                        lhsT=gS[:, 512:640],
                        rhs=wupS[:, 1, :], start=True, stop=True)
                    for b in range(4):
                        nc.tensor.matmul(
                            out=ftP[:, b, :],
                            lhsT=gS[:, b * 128:(b + 1) * 128],
                            rhs=wupS[:, min(b, 1), :],
                            start=b == 0, stop=b == 3)

                    # deferred epilogue of the previous chunk, issued into
                    # the middle of this chunk's PE stream
                    if u == 3 and prev is not None:
                        emit_epi(*prev)
                        prev = None

                    # evacuate only the tpw blocks; products read the
                    # feat blocks directly from PSUM bank 0 (different
                    # bank than the ACT read, so they run in parallel)
                    ftS = wp.tile([128, 8, 128], bf16, tag="ft")
                    nc.scalar.activation(out=ftS[:], in_=ftP[:, 0:8, :],
                                         func=COPYF)
                    featS = ftS[:, 0:4, :]
                    tpwS = ftS[:, 4:8, :]

                    # ---- elementwise product blocks (DVE, bf16) ----
                    # slots: p0, p1, p2x, p2y, p2z, q3; 6/7 scratch
                    prodS = wp.tile([128, 8, 128], bf16, tag="prod")
                    nc.vector.tensor_tensor(
                        out=prodS[:, 0:2, :], in0=tpwS[:, 0:2, :],
                        in1=featS[:, 0:1, :].broadcast_to([128, 2, 128]),
                        op=MUL)
                    nc.vector.tensor_tensor(
                        out=prodS[:, 2:5, :],
                        in0=tpwS[:, 2:3, :].broadcast_to([128, 3, 128]),
                        in1=featS[:, 1:4, :], op=MUL)
                    # q3 = t3 * (v.y1); the second operand comes straight
                    # from PSUM bank 3 of ftP (different bank than the cast)
                    nc.vector.tensor_tensor(
                        out=prodS[:, 5, :], in0=tpwS[:, 3, :],
                        in1=ftP[:, 8, :], op=MUL)

                    # ---- weighted segment-sum (product stationary) ----
                    # msgAB blocks [v1x, v1y, v1z, s1, v2x, v2y, v2z, s2]
                    nc.tensor.matmul(out=msgAB[:, 0:3, :],
                                     lhsT=prodS[:, 1, :],
                                     rhs=hS[:, NNODE:4 * NNODE],
                                     start=first, stop=False)
                    nc.tensor.matmul(out=msgAB[:, 3, :],
                                     lhsT=prodS[:, 0, :],
                                     rhs=hS[:, 0:NNODE],
                                     start=False, stop=False)
                    for m in range(3):
                        nc.tensor.matmul(
                            out=msgAB[:, 4 + m, :],
                            lhsT=prodS[:, 2 + m, :],
                            rhs=hS[:, 0:NNODE],
                            start=False, stop=False)
                    nc.tensor.matmul(out=msgAB[:, 7, :],
                                     lhsT=prodS[:, 5, :],
                                     rhs=hS[:, 4 * NNODE:5 * NNODE],
                                     start=False, stop=last)

                # copy accumulated messages to SBUF; epilogue deferred
                msgS = chp.tile([128, 8, NNODE], f32, tag="msgS")
                nc.vector.tensor_copy(out=msgS[:], in_=msgAB[:])
                prev = (k, msgS)

            emit_epi(*prev)

    nc.compile()
    return nc


# --------------------------------------------------------------------------
# Entry point
# --------------------------------------------------------------------------

def _assemble(results, per_core):
    out = np.zeros((N_NODES, 512), np.float32)
    for c in range(NCORES):
        ob = results[c]["outb"]
        for k, (s, e, _ec) in enumerate(per_core[c]["chunks"]):
            w = e - s
            if w == 0:
                continue
            rows = ob[k * NNODE: k * NNODE + w]
            out[s:e, :C] = rows[:, :C]
            out[s:e, C:] = np.stack(
                [rows[:, C:2 * C], rows[:, 2 * C:3 * C], rows[:, 3 * C:]],
                axis=2).reshape(w, 3 * C)
    return out


def run(inputs, trace=False, **kwargs):
    from concourse.bass_utils import run_bass_kernel_spmd

    per_core = _host_prep(inputs["node_feats"], inputs["edge_attrs"],
                          inputs["edge_feats"], inputs["edge_index"])
    wts = _weights_prep(inputs["W_up_s"], inputs["W_up_v"], inputs["W_mlp1"],
                        inputs["W_mlp2"], inputs["W_mlp3"], inputs["W_mlp4"],
                        inputs["W_lin_s"], inputs["W_lin_v"])
    in_maps = [
        {"gfeat": pc["gfeat"], "relidx": pc["relidx"], "y6": pc["y6"],
         "eft": pc["eft"], **wts}
        for pc in per_core
    ]
    nc = build_program()
    res = run_bass_kernel_spmd(nc, in_maps, core_ids=list(range(NCORES)),
                               trace=trace, **kwargs)
    return _assemble(res.results, per_core), res


def kernel(**inputs):
    return run(inputs)[0]


# revision 23
# speedup vs baseline: 1.2950x; 1.1018x over previous
"""MACE node-message block on 8 Trainium2 NeuronCores.

Strategy (receiver-sharded, no collectives):
  - Host sorts edges by receiver node and assigns each of the 8 cores a
    contiguous range of 1250 receiver nodes (+ its incoming edges).
  - Host gathers sender features per edge (np.take), transposes layouts,
    pads each core's edges into NCHUNK node-chunks (<=64 nodes) x TPC
    tiles of 128 edges.
  - The per-tile scatter matrices H[e, j*64+n] = y_j[e] * (rel[e]==n)
    (plus an unscaled one-hot block) are built ON DEVICE by GPSIMD
    local_scatter from (rel, y) -- 15 bytes/edge instead of 1KB dense H.
  - Device per tile (128 edges): up-project gathered feats + radial-MLP
    last layer into one 2-bank PSUM tile, single ACT evacuation, product
    blocks on DVE (with the m0b path pre-contracted: q3 = tpw3 * (v.y1)),
    then 6 PE matmuls accumulating messages [c, n] in a single PSUM bank
    per chunk (double-buffered across chunks so the PE never stalls).
  - Radial MLP runs the whole chunk (2x512 edges) packed to the full 128
    partitions via tile_position row/col tiling (full-width silu).
  - Chunk epilogues (output linear) are deferred into the middle of the
    NEXT chunk's tile loop, off the PE critical path.
  - Host reassembles the full [10000, 512] output.
"""

import numpy as np
import ml_dtypes

# ---- problem constants (hardcoded; kernel.py must be self-contained) ----
N_NODES = 10000
E_EDGES = 160000
C = 128
RB = 8
HID = 64
AVG_NEIGH = 16.0

C_000 = float(np.sqrt(0.5))
C_110 = float(np.sqrt(0.5) / np.sqrt(3.0))
C_011 = float(np.sqrt(1.5) / np.sqrt(3.0))
C_101 = float(np.sqrt(1.5) / np.sqrt(3.0))

NCORES = 8
NODES_PER_CORE = N_NODES // NCORES  # 1250
NCHUNK = 20            # node-chunks per core (<=NNODE nodes each)
NNODE = 64             # max nodes per chunk
TPC = 8                # tiles of 128 edges per chunk
CHUNK_SLOTS = TPC * 128   # 1024 edge slots per chunk
EPAD = NCHUNK * CHUNK_SLOTS
NTILES = NCHUNK * TPC
SUPER = 4              # tiles per supertile (512 edges)

BF16 = ml_dtypes.bfloat16


# --------------------------------------------------------------------------
# Host-side sharding / layout preparation
# --------------------------------------------------------------------------

def _host_prep(node_feats, edge_attrs, edge_feats, edge_index):
    sender = edge_index[0].astype(np.int64)
    receiver = edge_index[1].astype(np.int64)
    deg = np.bincount(receiver, minlength=N_NODES)
    order = np.argsort(receiver, kind="stable")
    node_edge_start = np.concatenate([[0], np.cumsum(deg)])

    per_core = []
    for c in range(NCORES):
        lo, hi = NODES_PER_CORE * c, NODES_PER_CORE * (c + 1)
        # greedy chunking: <=NNODE nodes and <=CHUNK_SLOTS edges per chunk
        chunks = []  # (node_start, node_end, edge_count)
        n = lo
        while n < hi:
            start = n
            ec = 0
            while n < hi and (n - start) < NNODE and ec + deg[n] <= CHUNK_SLOTS:
                ec += deg[n]
                n += 1
            chunks.append((start, n, int(ec)))
        assert len(chunks) <= NCHUNK, (
            f"core {c}: needs {len(chunks)} chunks > NCHUNK={NCHUNK}"
        )
        while len(chunks) < NCHUNK:
            chunks.append((hi, hi, 0))

        slot_sender = np.zeros(EPAD, np.int64)
        slot_rel = np.zeros(EPAD, np.int64)
        slot_y = np.zeros((EPAD, 4), np.float32)
        slot_ef = np.zeros((EPAD, RB), np.float32)
        used = np.zeros(EPAD, np.float32)
        for k, (s, e, ec) in enumerate(chunks):
            if ec == 0:
                continue
            seg = order[node_edge_start[s]: node_edge_start[s] + ec]
            base = k * CHUNK_SLOTS
            slot_sender[base: base + ec] = sender[seg]
            slot_rel[base: base + ec] = receiver[seg] - s
            slot_y[base: base + ec] = edge_attrs[seg]
            slot_ef[base: base + ec] = edge_feats[seg]
            used[base: base + ec] = 1.0

        # gathered sender feats, transposed per block: [t, cin, blk, e]
        # block 4 = gvy(e, cin) = sum_m v_m(e, cin) * y1m(e)  (host-folded,
        # like the y-scaled scatter matrices; Wv^T gvy = v . y1 up-projected)
        g = node_feats[slot_sender]                      # [EPAD, 512]
        s_blk = g[:, :C]
        v = g[:, C:].reshape(EPAD, C, 3)
        gvy = np.einsum('ecm,em->ec', v, slot_y[:, 1:4]).astype(np.float32)
        blocks = np.stack([s_blk, v[:, :, 0], v[:, :, 1], v[:, :, 2], gvy],
                          axis=1)
        gfeat = np.ascontiguousarray(
            blocks.reshape(NTILES, 128, 5, C).transpose(0, 3, 2, 1)
        ).astype(BF16)
        g4 = np.ascontiguousarray(
            gfeat.reshape(NTILES // SUPER, SUPER, 128, 5 * C)
            .transpose(0, 2, 1, 3))

        # H-build inputs: per chunk [128 part(e-in-tile), TPC, 6]
        # blocks j=0..3: H_j[e, j*64+rel] = y_j(e); block 4: unscaled
        # one-hot mask[e, 4*64+rel] = used(e); col 5 unused (-1).
        rel_t = slot_rel.reshape(NCHUNK, TPC, 128)      # [k, t, e]
        y_t = slot_y.reshape(NCHUNK, TPC, 128, 4)       # [k, t, e, 4]
        used_t = used.reshape(NCHUNK, TPC, 128)
        relidx = np.full((NCHUNK, TPC, 128, 6), -1, np.int16)
        relidx[:, :, :, :5] = (np.arange(5)[None, None, None, :] * NNODE
                               + rel_t[:, :, :, None])
        relidx = np.ascontiguousarray(relidx.transpose(0, 2, 1, 3))
        y6 = np.zeros((NCHUNK, TPC, 128, 6), np.float32)
        y6[:, :, :, :4] = y_t
        y6[:, :, :, 4] = used_t
        y6 = np.ascontiguousarray(y6.transpose(0, 2, 1, 3)).astype(BF16)

        # radial basis, transposed per chunk: [k, r, slot]
        eft = np.ascontiguousarray(
            slot_ef.reshape(NCHUNK, CHUNK_SLOTS, RB).transpose(0, 2, 1)
        ).astype(BF16)

        per_core.append({
            "chunks": chunks,
            "gfeat": g4,
            "relidx": relidx,
            "y6": y6,
            "eft": eft,
        })
    return per_core


def _weights_prep(W_up_s, W_up_v, W_mlp1, W_mlp2, W_mlp3, W_mlp4,
                  W_lin_s, W_lin_v):
    su = 1.0 / np.sqrt(np.float32(C))
    wup = np.stack([W_up_s * su, W_up_v * su], axis=1).astype(BF16)  # [cin,2,cout]

    w1 = (W_mlp1 / np.sqrt(np.float32(RB))).astype(BF16)
    # hidden-layer weights duplicated on both partition halves for the
    # tile_position-packed MLP (rows 0:64 and 64:128 hold the same matrix)
    w2d = np.concatenate([(W_mlp2 / np.sqrt(np.float32(HID)))] * 2,
                         axis=0).astype(BF16)           # [128, HID]
    w3d = np.concatenate([(W_mlp3 / np.sqrt(np.float32(HID)))] * 2,
                         axis=0).astype(BF16)
    # per-path constants folded into the last MLP layer (tpw order 0,1,2,3)
    scales = np.array([C_000, C_011, C_101, C_110], np.float32)
    w4p = ((W_mlp4 / np.sqrt(np.float32(HID))).reshape(HID, 4, C)
           * scales[None, :, None]).reshape(HID, 4 * C)
    w4pd = np.concatenate([w4p] * 2, axis=0).astype(BF16)  # [128, 4C]

    sl = 1.0 / (np.sqrt(np.float32(2 * C)) * AVG_NEIGH)
    wls = W_lin_s * sl   # [256, 128]
    wlv = W_lin_v * sl
    wl = np.stack([wls[:C], wls[C:], wlv[:C], wlv[C:]], axis=1).astype(np.float32)
    return {"wup": wup, "w1": w1, "w2d": w2d, "w3d": w3d, "w4pd": w4pd,
            "wl": wl}


# --------------------------------------------------------------------------
# Device program
# --------------------------------------------------------------------------

def build_program():
    import concourse.bacc as bacc
    import concourse.mybir as mybir
    import concourse.tile as tile

    f32 = mybir.dt.float32
    bf16 = mybir.dt.bfloat16
    i16 = mybir.dt.int16
    MUL = mybir.AluOpType.mult
    ADD = mybir.AluOpType.add

    nc = bacc.Bacc(None, target_bir_lowering=False)

    NSUPER = NTILES // SUPER
    gfeatD = nc.dram_tensor("gfeat", [NSUPER, 128, SUPER, 640], bf16,
                            kind="ExternalInput")
    relidxD = nc.dram_tensor("relidx", [NCHUNK, 128, TPC, 6], i16,
                             kind="ExternalInput")
    y6D = nc.dram_tensor("y6", [NCHUNK, 128, TPC, 6], bf16,
                         kind="ExternalInput")
    eftD = nc.dram_tensor("eft", [NCHUNK, RB, CHUNK_SLOTS], bf16,
                          kind="ExternalInput")
    wupD = nc.dram_tensor("wup", [128, 2, 128], bf16, kind="ExternalInput")
    w1D = nc.dram_tensor("w1", [RB, HID], bf16, kind="ExternalInput")
    w2dD = nc.dram_tensor("w2d", [128, HID], bf16, kind="ExternalInput")
    w3dD = nc.dram_tensor("w3d", [128, HID], bf16, kind="ExternalInput")
    w4pdD = nc.dram_tensor("w4pd", [128, 4 * C], bf16, kind="ExternalInput")
    wlD = nc.dram_tensor("wl", [128, 4, 128], f32, kind="ExternalInput")
    outD = nc.dram_tensor("outb", [NCHUNK * NNODE, 512], f32,
                          kind="ExternalOutput")

    SILU = mybir.ActivationFunctionType.Silu
    COPYF = mybir.ActivationFunctionType.Copy

    with tile.TileContext(nc) as tc:
        with (
            tc.tile_pool(name="const", bufs=1) as cp,
            tc.tile_pool(name="work", bufs=4) as wp,
            tc.tile_pool(name="hpool", bufs=4) as hp,
            tc.tile_pool(name="hs", bufs=2) as hsp,
            tc.tile_pool(name="chk", bufs=2) as chp,
            tc.tile_pool(name="ps", bufs=2, space="PSUM") as ps,
            tc.tile_pool(name="misc", bufs=1, space="PSUM") as mp,
            tc.tile_pool(name="msgp", bufs=1, space="PSUM") as msgp,
        ):
            # constants
            wupS = cp.tile([128, 2, 128], bf16, tag="wup")
            w1S = cp.tile([RB, HID], bf16, tag="w1")
            w2dS = cp.tile([128, HID], bf16, tag="w2d")
            w3dS = cp.tile([128, HID], bf16, tag="w3d")
            w4pdS = cp.tile([128, 4 * C], bf16, tag="w4pd")
            wlS = cp.tile([128, 4, 128], f32, tag="wl")
            nc.sync.dma_start(out=wupS[:], in_=wupD[:])
            nc.sync.dma_start(out=w1S[:], in_=w1D[:])
            nc.sync.dma_start(out=w2dS[:], in_=w2dD[:])
            nc.sync.dma_start(out=w3dS[:], in_=w3dD[:])
            nc.sync.dma_start(out=w4pdS[:], in_=w4pdD[:])
            nc.sync.dma_start(out=wlS[:], in_=wlD[:])

            def emit_epi(k, msgS):
                # output linear for chunk k from its SBUF message copy.
                # msgS blocks: [v1x, v1y, v1z, s1, v2x, v2y, v2z, s2]
                outPF = mp.tile([128, 512], f32, tag="scr")
                outP = outPF[0:NNODE, :]
                nc.tensor.matmul(out=outP[:, 0:128], lhsT=msgS[:, 3, :],
                                 rhs=wlS[:, 0, :], start=True, stop=False)
                nc.tensor.matmul(out=outP[:, 0:128], lhsT=msgS[:, 7, :],
                                 rhs=wlS[:, 1, :], start=False, stop=False)
                for m in range(3):
                    osl = slice((1 + m) * 128, (2 + m) * 128)
                    nc.tensor.matmul(out=outP[:, osl],
                                     lhsT=msgS[:, m, :],
                                     rhs=wlS[:, 2, :], start=False,
                                     stop=False)
                    nc.tensor.matmul(out=outP[:, osl],
                                     lhsT=msgS[:, 4 + m, :],
                                     rhs=wlS[:, 3, :], start=False,
                                     stop=m == 2)
                outS = chp.tile([NNODE, 512], f32, tag="outS")
                nc.vector.tensor_copy(out=outS[:], in_=outP[:])
                nc.sync.dma_start(out=outD[k * NNODE:(k + 1) * NNODE, :],
                                  in_=outS[:])

            prev = None  # (k, msgS) awaiting epilogue
            for k in range(NCHUNK):
                efS = chp.tile([RB, CHUNK_SLOTS], bf16, tag="ef")
                nc.sync.dma_start(out=efS[:], in_=eftD[k])
                relS = chp.tile([128, TPC, 6], i16, tag="rel")
                nc.sync.dma_start(out=relS[:], in_=relidxD[k])
                y6S = chp.tile([128, TPC, 6], bf16, tag="y6")
                nc.sync.dma_start(out=y6S[:], in_=y6D[k])

                # message accumulator, [cin-block, n] orientation, 1 bank:
                # blocks [v1x, v1y, v1z, s1, v2x, v2y, v2z, s2]
                msgAB = msgp.tile([128, 8, NNODE], f32, tag="msgAB")

                # ---- radial MLP: whole chunk (2x512 edges) packed to 128
                # partitions via tile_position row/col tiling ----
                ea = slice(0, 512)
                eb = slice(512, 1024)
                h1p = mp.tile([128, 512], f32, tag="scr")
                nc.tensor.matmul(out=h1p[0:64, :], lhsT=w1S[:],
                                 rhs=efS[:, ea], start=True, stop=True,
                                 tile_position=(0, 0))
                nc.tensor.matmul(out=h1p[64:128, :], lhsT=w1S[:],
                                 rhs=efS[:, eb], start=True, stop=True,
                                 tile_position=(0, 64))
                h1s = hsp.tile([128, 512], bf16, tag="h1s")
                nc.scalar.activation(out=h1s[:], in_=h1p[:], func=SILU)

                h2p = mp.tile([128, 512], f32, tag="scr")
                nc.tensor.matmul(out=h2p[0:64, :], lhsT=w2dS[0:64, :],
                                 rhs=h1s[0:64, :], start=True, stop=True,
                                 tile_position=(0, 0))
                nc.tensor.matmul(out=h2p[64:128, :], lhsT=w2dS[64:128, :],
                                 rhs=h1s[64:128, :], start=True,
                                 stop=True, tile_position=(64, 64))
                h2s = hsp.tile([128, 512], bf16, tag="h2s")
                nc.scalar.activation(out=h2s[:], in_=h2p[:], func=SILU)

                h3p = mp.tile([128, 512], f32, tag="scr")
                nc.tensor.matmul(out=h3p[0:64, :], lhsT=w3dS[0:64, :],
                                 rhs=h2s[0:64, :], start=True, stop=True,
                                 tile_position=(0, 0))
                nc.tensor.matmul(out=h3p[64:128, :], lhsT=w3dS[64:128, :],
                                 rhs=h2s[64:128, :], start=True,
                                 stop=True, tile_position=(64, 64))
                h3s = hsp.tile([128, 512], bf16, tag="h3s")
                nc.scalar.activation(out=h3s[:], in_=h3p[:], func=SILU)

                for u in range(TPC):
                    sg = k * 2 + u // SUPER         # global supertile index
                    uu = u % SUPER                  # tile within supertile
                    half = u // SUPER               # which partition half
                    first = u == 0
                    last = u == TPC - 1

                    if uu == 0:
                        g4 = wp.tile([128, SUPER, 640], bf16, tag="g")
                        nc.sync.dma_start(out=g4[:], in_=gfeatD[sg])
                    gS = g4[:, uu, :]

                    # ---- H build on GPSIMD: H[e, j*64+rel] = y_j, plus
                    # unscaled one-hot mask in block 4 ----
                    hS = hp.tile([128, 5 * NNODE], bf16, tag="hm")
                    nc.gpsimd.local_scatter(
                        out_ap=hS[:], data_ap=y6S[:, u, :],
                        idxs_ap=relS[:, u, :], channels=128,
                        num_elems=5 * NNODE, num_idxs=6)

                    # ---- up-projection + tpw into one 3-bank PSUM tile:
                    # cols 0:512 feat (S, V1..V3), 512:1024 tpw (t0..t3),
                    # 1024:1152 VD = Wv^T gvy (= v.y1 up-projected).
                    # tpw first so its ACT evacuation overlaps the up MMs.
                    ftP = ps.tile([128, 9, 128], f32, tag="ftp")
                    for b in range(4):
                        nc.tensor.matmul(
                            out=ftP[:, b, :],
                            lhsT=gS[:, b * 128:(b + 1) * 128],
                            rhs=wupS[:, min(b, 1), :],
                            start=b == 0, stop=b == 3)
                    nc.tensor.matmul(
                        out=ftP[:, 8, :],
                        lhsT=gS[:, 512:640],
                        rhs=wupS[:, 1, :], start=True, stop=True)
                    hrow = slice(64 * half, 64 * half + 64)
                    nc.tensor.matmul(
                        out=ftP[:, 4:8, :],
                        lhsT=h3s[hrow, uu * 128:(uu + 1) * 128],
                        rhs=w4pdS[hrow, :], start=True, stop=True)

                    # deferred epilogue of the previous chunk, issued into
                    # the middle of this chunk's PE stream
                    if u == 3 and prev is not None:
                        emit_epi(*prev)
                        prev = None

                    # evacuate only the tpw blocks; products read the
                    # feat blocks directly from PSUM bank 0 (different
                    # bank than the ACT read, so they run in parallel)
                    ftS = wp.tile([128, 8, 128], bf16, tag="ft")
                    nc.scalar.activation(out=ftS[:], in_=ftP[:, 0:8, :],
                                         func=COPYF)
                    featS = ftS[:, 0:4, :]
                    tpwS = ftS[:, 4:8, :]

                    # ---- elementwise product blocks (DVE, bf16) ----
                    # slots: p0, p1, p2x, p2y, p2z, q3; 6/7 scratch
                    prodS = wp.tile([128, 8, 128], bf16, tag="prod")
                    nc.vector.tensor_tensor(
                        out=prodS[:, 0:2, :], in0=tpwS[:, 0:2, :],
                        in1=featS[:, 0:1, :].broadcast_to([128, 2, 128]),
                        op=MUL)
                    nc.vector.tensor_tensor(
                        out=prodS[:, 2:5, :],
                        in0=tpwS[:, 2:3, :].broadcast_to([128, 3, 128]),
                        in1=featS[:, 1:4, :], op=MUL)
                    # q3 = t3 * (v.y1); the second operand comes straight
                    # from PSUM bank 3 of ftP (different bank than the cast)
                    nc.vector.tensor_tensor(
                        out=prodS[:, 5, :], in0=tpwS[:, 3, :],
                        in1=ftP[:, 8, :], op=MUL)

                    # ---- weighted segment-sum (product stationary) ----
                    # msgAB blocks [v1x, v1y, v1z, s1, v2x, v2y, v2z, s2]
                    nc.tensor.matmul(out=msgAB[:, 0:3, :],
                                     lhsT=prodS[:, 1, :],
                                     rhs=hS[:, NNODE:4 * NNODE],
                                     start=first, stop=False)
                    nc.tensor.matmul(out=msgAB[:, 3, :],
                                     lhsT=prodS[:, 0, :],
                                     rhs=hS[:, 0:NNODE],
                                     start=False, stop=False)
                    for m in range(3):
                        nc.tensor.matmul(
                            out=msgAB[:, 4 + m, :],
                            lhsT=prodS[:, 2 + m, :],
                            rhs=hS[:, 0:NNODE],
                            start=False, stop=False)
                    nc.tensor.matmul(out=msgAB[:, 7, :],
                                     lhsT=prodS[:, 5, :],
                                     rhs=hS[:, 4 * NNODE:5 * NNODE],
                                     start=False, stop=last)

                # copy accumulated messages to SBUF; epilogue deferred
                msgS = chp.tile([128, 8, NNODE], f32, tag="msgS")
                nc.vector.tensor_copy(out=msgS[:], in_=msgAB[:])
                prev = (k, msgS)

            emit_epi(*prev)

    nc.compile()
    return nc


# --------------------------------------------------------------------------
# Entry point
# --------------------------------------------------------------------------

def _assemble(results, per_core):
    out = np.zeros((N_NODES, 512), np.float32)
    for c in range(NCORES):
        ob = results[c]["outb"]
        for k, (s, e, _ec) in enumerate(per_core[c]["chunks"]):
            w = e - s
            if w == 0:
                continue
            rows = ob[k * NNODE: k * NNODE + w]
            out[s:e, :C] = rows[:, :C]
            out[s:e, C:] = np.stack(
                [rows[:, C:2 * C], rows[:, 2 * C:3 * C], rows[:, 3 * C:]],
                axis=2).reshape(w, 3 * C)
    return out


def run(inputs, trace=False, **kwargs):
    from concourse.bass_utils import run_bass_kernel_spmd

    per_core = _host_prep(inputs["node_feats"], inputs["edge_attrs"],
                          inputs["edge_feats"], inputs["edge_index"])
    wts = _weights_prep(inputs["W_up_s"], inputs["W_up_v"], inputs["W_mlp1"],
                        inputs["W_mlp2"], inputs["W_mlp3"], inputs["W_mlp4"],
                        inputs["W_lin_s"], inputs["W_lin_v"])
    in_maps = [
        {"gfeat": pc["gfeat"], "relidx": pc["relidx"], "y6": pc["y6"],
         "eft": pc["eft"], **wts}
        for pc in per_core
    ]
    nc = build_program()
    res = run_bass_kernel_spmd(nc, in_maps, core_ids=list(range(NCORES)),
                               trace=trace, **kwargs)
    return _assemble(res.results, per_core), res


def kernel(**inputs):
    return run(inputs)[0]
